# revision 5
# baseline (speedup 1.0000x reference)
"""Trainium2 Bass kernel for nn_BodyAvgDiseaseFeatureAttn2.

Computation (reference):
    attn  = softmax over channels of [heart(27); lung(28); lung(28)] -> [83, 16]
    Weff[o,c,h,w] = attn[o,c] * Wfc[o,c,h,w]
    out[b,o] = mean_s( sum_{c,h,w} x[b,s,c,h,w] * Weff[o,c,h,w] ) + bias[o]

Kernel strategy (pure data parallel, 8 cores, batch-sharded):
  The tiny parameter math (softmax, attention*Wfc fuse, 1/S fold, chunk
  transposes) happens on the host. x is shipped per-core as bf16 in a
  [ck=576, s=15, b=512] layout (plus one all-ones row), so the
  contraction axis (ck) is the partition axis and each SBUF partition
  line is one 15360-byte contiguous DRAM run -- the whole input streams
  in as five ~1.8 MB DMAs at near-peak descriptor efficiency.

  The slice-mean AND the bias fold into the matmul: with the
  per-disease weight W2[ck] = Weff[:, ck]/S stationary (ones-row weight
  = bias/S), the 15 s-slabs of a ck-chunk are 15 accumulating matmuls
  (N=512) into one PSUM bank, so PE does the s-sum, the FC contraction
  and the bias in one pass: 75 matmuls per core, no transposes, no DVE
  reduction tree, no DMA-accumulate chains.

  Input DMAs alternate between the sync (HWDGE) and gpsimd (SWDGE)
  queues while the output store runs on scalar, so each sequencer's
  in-order stream lets iteration i+1's loads issue while iteration i
  computes; the weight tile is loaded once, outside the repeat loop.
"""

import numpy as np
import ml_dtypes
from contextlib import ExitStack

import concourse.bass as bass
import concourse.bacc as bacc
import concourse.tile as tile
import concourse.mybir as mybir
from concourse.bass_utils import run_bass_kernel_spmd

F32 = mybir.dt.float32
BF16 = mybir.dt.bfloat16

N_CORES = 8
B, S, C, H, W = 4096, 15, 16, 6, 6
CK = C * H * W            # 576
CKP = CK + 1              # 577: +1 all-ones row carrying the bias
BS = B // N_CORES         # 512 volumes per core
SBS = S * BS              # 7680 columns per ck row
NH, NL = 27, 28
O = 2 * NL + NH           # 83
KC = [128, 128, 128, 128, 65]  # ck chunking of 577
NK = len(KC)


def _build_body(ctx, tc, o_d, x_d, wv, xp, pout, osb):
    nc = tc.nc

    xts = []
    for t, kw in enumerate(KC):
        xt = xp.tile([128, SBS], BF16, tag="xt", name="xt")
        q = nc.sync if t % 2 == 0 else nc.scalar
        q.dma_start(xt[0:kw, :], x_d[t * 128:t * 128 + kw, :])
        xts.append(xt)

    po = pout.tile([O, BS], F32, tag="po", name="po")
    for t, kw in enumerate(KC):
        for j in range(S):
            nc.tensor.matmul(po[:, :], wv[0:kw, t * O:(t + 1) * O],
                             xts[t][0:kw, j * BS:(j + 1) * BS],
                             start=(t == 0 and j == 0),
                             stop=(t == NK - 1 and j == S - 1))
    outsb = osb.tile([O, BS], F32, tag="outsb", name="outsb")
    nc.vector.tensor_copy(outsb[:], po[:])
    nc.gpsimd.dma_start(o_d[:, :], outsb[:])


def build_program(repeat: int = 1):
    nc = bacc.Bacc("TRN2", target_bir_lowering=False, debug=False,
                   num_devices=N_CORES)
    x_d = nc.dram_tensor("xt2", [CKP, SBS], BF16, kind="ExternalInput").ap()
    w_d = nc.dram_tensor("wv", [128, NK * O], BF16, kind="ExternalInput").ap()
    o_d = nc.dram_tensor("out", [O, BS], F32, kind="ExternalOutput").ap()

    with tile.TileContext(nc) as tc:
        with ExitStack() as ctx:
            const = ctx.enter_context(tc.tile_pool(name="const", bufs=1))
            xp = ctx.enter_context(tc.tile_pool(name="xp", bufs=10))
            pout = ctx.enter_context(
                tc.tile_pool(name="pout", bufs=2, space="PSUM"))
            osb = ctx.enter_context(tc.tile_pool(name="osb", bufs=4))

            wv = const.tile([128, NK * O], BF16)
            nc.scalar.dma_start(wv[:], w_d[:, :])

            if repeat == 1:
                _build_body(ctx, tc, o_d, x_d, wv, xp, pout, osb)
            else:
                with tc.For_i(0, repeat, 1, staggered_reset=True) as _iv:
                    _build_body(ctx, tc, o_d, x_d, wv, xp, pout, osb)
    nc.compile()
    return nc


_NC_CACHE = {}


def _get_program(repeat: int = 1):
    if repeat not in _NC_CACHE:
        _NC_CACHE[repeat] = build_program(repeat)
    return _NC_CACHE[repeat]


def _host_pack(inputs):
    """Fuse softmax attention into the FC weights, fold 1/S, append the
    bias row, chunk and transpose into the [128, 5*83] bf16 layout."""
    h = np.asarray(inputs["dzfeatweights_heart"], np.float32).reshape(NH, C)
    l = np.asarray(inputs["dzfeatweights_lung"], np.float32).reshape(NL, C)
    att = np.concatenate([h, l, l], axis=0)
    att = np.exp(att - att.max(axis=1, keepdims=True))
    att = att / att.sum(axis=1, keepdims=True) / S
    wfc = np.asarray(inputs["fclayers_weights"], np.float32).reshape(O, C, H * W)
    weff = (att[:, :, None] * wfc).reshape(O, CK)
    bias = np.asarray(inputs["fclayers_biases"], np.float32).reshape(O, 1)
    weffp = np.concatenate([weff, bias / S], axis=1)       # [83, 577]
    wv = np.zeros((128, NK * O), np.float32)
    c0 = 0
    for t, kw in enumerate(KC):
        wv[0:kw, t * O:(t + 1) * O] = weffp[:, c0:c0 + kw].T
        c0 += kw
    return wv.astype(ml_dtypes.bfloat16)


def make_in_maps(inputs):
    x = np.asarray(inputs["x"], dtype=np.float32).reshape(B, S, CK)
    wv = _host_pack(inputs)
    maps = []
    for c in range(N_CORES):
        xc = x[c * BS:(c + 1) * BS]                        # [512, 15, 576]
        xt2 = np.empty((CKP, S * BS), ml_dtypes.bfloat16)
        xt2[0:CK] = np.ascontiguousarray(
            xc.transpose(2, 1, 0)).astype(ml_dtypes.bfloat16).reshape(CK, SBS)
        xt2[CK] = np.ones(SBS, ml_dtypes.bfloat16)
        maps.append({"xt2": xt2, "wv": wv})
    return maps


def assemble_output(results):
    outs = [results[c]["out"] for c in range(N_CORES)]    # each [83, 512]
    return np.ascontiguousarray(np.concatenate(outs, axis=1).T)  # [4096, 83]


def kernel(**inputs) -> np.ndarray:
    nc = _get_program(1)
    in_maps = make_in_maps(inputs)
    res = run_bass_kernel_spmd(nc, in_maps, core_ids=list(range(N_CORES)))
    return assemble_output(res.results)


# revision 6
# speedup vs baseline: 2.4269x; 2.4269x over previous
"""Trainium2 Bass kernel for nn_BodyAvgDiseaseFeatureAttn2.

Computation (reference):
    attn  = softmax over channels of [heart(27); lung(28); lung(28)] -> [83, 16]
    Weff[o,c,h,w] = attn[o,c] * Wfc[o,c,h,w]
    out[b,o] = mean_s( sum_{c,h,w} x[b,s,c,h,w] * Weff[o,c,h,w] ) + bias[o]

Kernel strategy (pure data parallel, 8 cores, batch-sharded):
  The tiny parameter math (softmax, attention*Wfc fuse, 1/S fold, chunk
  transposes) happens on the host. x is shipped per-core as bf16 in a
  [ck=576, s=15, b=512] layout (plus one all-ones row), so the
  contraction axis (ck) is the partition axis and each SBUF partition
  line is one 15360-byte contiguous DRAM run -- the whole input streams
  in as five ~1.8 MB DMAs at near-peak descriptor efficiency.

  The slice-mean AND the bias fold into the matmul: with the
  per-disease weight W2[ck] = Weff[:, ck]/S stationary (ones-row weight
  = bias/S), the 15 s-slabs of a ck-chunk are 15 accumulating matmuls
  (N=512) into one PSUM bank, so PE does the s-sum, the FC contraction
  and the bias in one pass: 75 matmuls per core, no transposes, no DVE
  reduction tree, no DMA-accumulate chains.

  Input DMAs alternate between the sync (HWDGE) and gpsimd (SWDGE)
  queues while the output store runs on scalar, so each sequencer's
  in-order stream lets iteration i+1's loads issue while iteration i
  computes; the weight tile is loaded once, outside the repeat loop.
"""

import numpy as np
import ml_dtypes
from contextlib import ExitStack

import concourse.bass as bass
import concourse.bacc as bacc
import concourse.tile as tile
import concourse.mybir as mybir
from concourse.bass_utils import run_bass_kernel_spmd

F32 = mybir.dt.float32
BF16 = mybir.dt.bfloat16

N_CORES = 8
B, S, C, H, W = 4096, 15, 16, 6, 6
CK = C * H * W            # 576
CKP = CK + 1              # 577: +1 all-ones row carrying the bias
BS = B // N_CORES         # 512 volumes per core
SBS = S * BS              # 7680 columns per ck row
NH, NL = 27, 28
O = 2 * NL + NH           # 83
KC = [128, 128, 128, 128, 65]  # ck chunking of 577
NK = len(KC)


def _build_body(ctx, tc, o_d, x_d, wv, xp, pout, osb):
    nc = tc.nc

    xts = []
    for t, kw in enumerate(KC):
        xt = xp.tile([128, SBS], BF16, tag="xt", name="xt")
        q = nc.sync if t % 2 == 0 else nc.scalar
        q.dma_start(xt[0:kw, :], x_d[t * 128:t * 128 + kw, :])
        xts.append(xt)

    po = pout.tile([O, BS], F32, tag="po", name="po")
    for t, kw in enumerate(KC):
        for j in range(S):
            nc.tensor.matmul(po[:, :], wv[0:kw, t * O:(t + 1) * O],
                             xts[t][0:kw, j * BS:(j + 1) * BS],
                             start=(t == 0 and j == 0),
                             stop=(t == NK - 1 and j == S - 1))
    outsb = osb.tile([O, BS], F32, tag="outsb", name="outsb")
    nc.vector.tensor_copy(outsb[:], po[:])
    nc.gpsimd.dma_start(o_d[:, :], outsb[:])


def build_program(repeat: int = 1):
    nc = bacc.Bacc("TRN2", target_bir_lowering=False, debug=False,
                   num_devices=N_CORES)
    x_d = nc.dram_tensor("xt2", [CKP, SBS], BF16, kind="ExternalInput").ap()
    w_d = nc.dram_tensor("wv", [128, NK * O], BF16, kind="ExternalInput").ap()
    o_d = nc.dram_tensor("out", [O, BS], F32, kind="ExternalOutput").ap()

    with tile.TileContext(nc) as tc:
        with ExitStack() as ctx:
            const = ctx.enter_context(tc.tile_pool(name="const", bufs=1))
            xp = ctx.enter_context(tc.tile_pool(name="xp", bufs=10))
            pout = ctx.enter_context(
                tc.tile_pool(name="pout", bufs=2, space="PSUM"))
            osb = ctx.enter_context(tc.tile_pool(name="osb", bufs=4))

            wv = const.tile([128, NK * O], BF16)
            nc.scalar.dma_start(wv[:], w_d[:, :])

            if repeat == 1:
                _build_body(ctx, tc, o_d, x_d, wv, xp, pout, osb)
            else:
                def body(_iv):
                    _build_body(ctx, tc, o_d, x_d, wv, xp, pout, osb)
                tc.For_i_unrolled(0, repeat, 1, body, max_unroll=16)
    nc.compile()
    return nc


_NC_CACHE = {}


def _get_program(repeat: int = 1):
    if repeat not in _NC_CACHE:
        _NC_CACHE[repeat] = build_program(repeat)
    return _NC_CACHE[repeat]


def _host_pack(inputs):
    """Fuse softmax attention into the FC weights, fold 1/S, append the
    bias row, chunk and transpose into the [128, 5*83] bf16 layout."""
    h = np.asarray(inputs["dzfeatweights_heart"], np.float32).reshape(NH, C)
    l = np.asarray(inputs["dzfeatweights_lung"], np.float32).reshape(NL, C)
    att = np.concatenate([h, l, l], axis=0)
    att = np.exp(att - att.max(axis=1, keepdims=True))
    att = att / att.sum(axis=1, keepdims=True) / S
    wfc = np.asarray(inputs["fclayers_weights"], np.float32).reshape(O, C, H * W)
    weff = (att[:, :, None] * wfc).reshape(O, CK)
    bias = np.asarray(inputs["fclayers_biases"], np.float32).reshape(O, 1)
    weffp = np.concatenate([weff, bias / S], axis=1)       # [83, 577]
    wv = np.zeros((128, NK * O), np.float32)
    c0 = 0
    for t, kw in enumerate(KC):
        wv[0:kw, t * O:(t + 1) * O] = weffp[:, c0:c0 + kw].T
        c0 += kw
    return wv.astype(ml_dtypes.bfloat16)


def make_in_maps(inputs):
    x = np.asarray(inputs["x"], dtype=np.float32).reshape(B, S, CK)
    wv = _host_pack(inputs)
    maps = []
    for c in range(N_CORES):
        xc = x[c * BS:(c + 1) * BS]                        # [512, 15, 576]
        xt2 = np.empty((CKP, S * BS), ml_dtypes.bfloat16)
        xt2[0:CK] = np.ascontiguousarray(
            xc.transpose(2, 1, 0)).astype(ml_dtypes.bfloat16).reshape(CK, SBS)
        xt2[CK] = np.ones(SBS, ml_dtypes.bfloat16)
        maps.append({"xt2": xt2, "wv": wv})
    return maps


def assemble_output(results):
    outs = [results[c]["out"] for c in range(N_CORES)]    # each [83, 512]
    return np.ascontiguousarray(np.concatenate(outs, axis=1).T)  # [4096, 83]


def kernel(**inputs) -> np.ndarray:
    nc = _get_program(1)
    in_maps = make_in_maps(inputs)
    res = run_bass_kernel_spmd(nc, in_maps, core_ids=list(range(N_CORES)))
    return assemble_output(res.results)
